# revision 27
# baseline (speedup 1.0000x reference)
"""DSAutoCorrelation Trainium2 kernel (v5).

Math (B=16, L=2048, H=8, E=64, C=H*E=512, top_k=7):
  R[b,l]    = sum_t <k[b,t,:], q[b,(t+l)%L,:]>_c      (= C * mean_value[b,l])
  topk over mean_b R -> 7 delays d_k; w[b,:] = softmax(R[b,d]/C)
  out[b,l,:] = sum_k w[b,k] * v[b,(l+d_k)%L,:]

Device split (8 cores, 2 batches each):
  K1 (static): D[b,p,u] = sum_{i<16,c} K^T[c,128i+p] * Q^T[c,(128i+u)%L]
      fp8 e4m3 matmuls in DoubleRow perf mode (2 channel-blocks packed per
      matmul, ~2 moving elems/cycle).  D is used ONLY for the top-k delay
      selection (fp8 noise is ~50x below the rank-7/8 margin for gaussian
      data); the 7 selected softmax logits are recomputed exactly on the
      host (0.01% of the FLOPs), so the weights carry no fp8 error.
      Wraparound via split matmuls.  kt issues on SP queue, qt on ACT
      queue; the first compute-critical pieces (kt[:, :128], qt[:, :512])
      are split out as small leading DMAs so the first matmul is not
      gated on full-tile transfers.
  K2 (lazy-compiled per delay set — delays are global, one SPMD program):
      out^T[c,l] = sum_k w_k v^T[c,(l+d_k)%L] in transposed layout.
      Three engines: PE does diag(w) matmuls for 11264 of 16384 columns
      (ACT drains PSUM pairs 1024 wide), DVE runs 4-tap stt chains for the
      other 5120, and ACT additionally computes tap 0 (Copy with
      per-partition scale) plus two pre-scaled taps that DVE folds in with
      2x-packed bf16 tensor_tensor adds (scratches are written
      col-0-aligned by ACT so the DVE adds always hit the 4B-aligned 2x
      fast path).  The diag(w) stationaries are built on-device by the
      startup-idle DVE from a [128,128] identity (64x less input DMA).
      All inputs ride the SP ring in need-order ((0,3) is a split group so
      the PE's first work needs only the first vt tile); DVE-group output
      DMAs also issue on SP so they never block ACT work, and K2_SEQ
      orders the in-order ACT queue by expected ready-time.
"""

import numpy as np

B, L, H, E = 16, 2048, 8, 64
C = H * E
NCORES = 8
BPC = B // NCORES
TOPK = 7  # int(math.log(2048))
NB = L // 128  # 16 row-blocks

_CACHE = {}


def _build_k1():
    from concourse import bacc, mybir
    from concourse.tile import TileContext

    f32 = mybir.dt.float32
    f16 = mybir.dt.float16
    f8 = mybir.dt.float8e4
    DR = mybir.MatmulPerfMode.DoubleRow
    nc = bacc.Bacc("TRN2", target_bir_lowering=False, debug=False, num_devices=NCORES)
    qt = nc.dram_tensor("qt", (BPC, C, L), f8, kind="ExternalInput")
    kt = nc.dram_tensor("kt", (BPC, C, L), f8, kind="ExternalInput")
    Dout = nc.dram_tensor("D", (BPC, 128, L), f16, kind="ExternalOutput")

    with TileContext(nc) as tc:
        with (
            tc.tile_pool(name="qk", bufs=2) as qkpool,
            tc.tile_pool(name="ps", bufs=2, space="PSUM") as pspool,
            tc.tile_pool(name="dsb", bufs=4) as dpool,
        ):
            for b in range(BPC):
                kts = []
                qts = []
                # one [128, 2, L] tile per channel-block pair; kt issues on
                # SP, qt on ACT.  For the very first pair the leading 128
                # (kt) / 512 (qt) columns go out as their own small DMAs so
                # the first LDWEIGHTS/matmul deps land early.
                for pr in range(2):
                    kt_t = qkpool.tile([128, 2, L], f8, tag=f"kt{pr}", name=f"kt{pr}")
                    qt_t = qkpool.tile([128, 2, L], f8, tag=f"qt{pr}", name=f"qt{pr}")
                    if b == 0 and pr == 0:
                        # the first i-iteration consumes ALL of qt pair 0
                        # (one 512-wide window per u) but only kt[:, :128],
                        # so stage pieces in consumption order: tiny kt/qt
                        # leads, then qt window-by-window ahead of kt bulk
                        # leads all on the SP ring — the ACT ring's first
                        # transfer starts ~1.6us later than SP's, and
                        # per-ring bandwidth is only ~110GB/s, so the first
                        # (256-wide-split) matmul's deps are kept to 96KB
                        for j in range(2):
                            rows = slice(128 * j, 128 * (j + 1))
                            nc.sync.dma_start(kt_t[:, j, 0:128], kt[b, rows, 0:128])
                            nc.sync.dma_start(qt_t[:, j, 0:256], qt[b, rows, 0:256])
                        for j in range(2):
                            rows = slice(128 * j, 128 * (j + 1))
                            nc.sync.dma_start(qt_t[:, j, 256:512], qt[b, rows, 256:512])
                            nc.sync.dma_start(kt_t[:, j, 128:512], kt[b, rows, 128:512])
                            nc.scalar.dma_start(qt_t[:, j, 512:1024], qt[b, rows, 512:1024])
                        for j in range(2):
                            rows = slice(128 * j, 128 * (j + 1))
                            nc.scalar.dma_start(qt_t[:, j, 1024:1536], qt[b, rows, 1024:1536])
                            nc.scalar.dma_start(qt_t[:, j, 1536:L], qt[b, rows, 1536:L])
                            nc.sync.dma_start(kt_t[:, j, 512:L], kt[b, rows, 512:L])
                    else:
                        for j in range(2):
                            rows = slice(256 * pr + 128 * j, 256 * pr + 128 * (j + 1))
                            nc.sync.dma_start(kt_t[:, j, :], kt[b, rows, :])
                            nc.scalar.dma_start(qt_t[:, j, :], qt[b, rows, :])
                    kts.append(kt_t)
                    qts.append(qt_t)

                psums = [pspool.tile([128, 512], f32, tag=f"ps{u}", name=f"ps{u}") for u in range(4)]

                def mm(u, lhs, pr, i, first, last):
                    s = (128 * i + 512 * u) % L
                    if b == 0 and pr == 0 and i == 0 and u == 0:
                        # the chronologically first matmul: 256-wide halves
                        # (both start=True, disjoint psum cols) so it fires
                        # on a 96KB lead instead of waiting for qt[0:512]
                        for h in range(2):
                            hs = slice(256 * h, 256 * (h + 1))
                            nc.tensor.matmul(
                                psums[u][:, hs], lhs, qts[pr][:, :, hs],
                                start=first, stop=last, perf_mode=DR)
                    elif s + 512 <= L:
                        nc.tensor.matmul(
                            psums[u][:, 0:512], lhs, qts[pr][:, :, s:s + 512],
                            start=first, stop=last, perf_mode=DR)
                    else:
                        n1 = L - s
                        nc.tensor.matmul(
                            psums[u][:, 0:n1], lhs, qts[pr][:, :, s:L],
                            start=first, stop=last, perf_mode=DR)
                        nc.tensor.matmul(
                            psums[u][:, n1:512], lhs, qts[pr][:, :, 0:512 - n1],
                            start=first, stop=last, perf_mode=DR)

                # pair 0: u-inner; pair 1: u-outer with per-u stop so each
                # psum bank drains under the next u's matmuls
                for i in range(NB):
                    lhs = kts[0][:, :, 128 * i:128 * (i + 1)]
                    for u in range(4):
                        mm(u, lhs, 0, i, i == 0, False)
                for u in range(4):
                    for i in range(NB):
                        lhs = kts[1][:, :, 128 * i:128 * (i + 1)]
                        mm(u, lhs, 1, i, False, i == NB - 1)
                    d_sb = dpool.tile([128, 512], f16, tag="dsb", name="dsb")
                    if b == BPC - 1 and u == 3:
                        # the very last drain is tail-exposed: split it so
                        # the first half's DMA overlaps the second's CAST
                        for h in range(2):
                            hs = slice(256 * h, 256 * (h + 1))
                            nc.vector.tensor_copy(d_sb[:, hs], psums[u][:, hs])
                            nc.scalar.dma_start(
                                Dout[b, :, 512 * u + 256 * h:512 * u + 256 * (h + 1)],
                                d_sb[:, hs])
                    else:
                        nc.vector.tensor_copy(d_sb[:], psums[u][:])
                        nc.scalar.dma_start(Dout[b, :, 512 * u:512 * (u + 1)], d_sb[:])
    nc.compile()
    return nc


# k2 per-group column splits: group (b,cc) -> SPL; cols [0:SPL) go to the
# DVE/ACT chain, [SPL:L) to the PE (must be a multiple of 512).  (0,3) is
# split so the PE's very first work only needs the FIRST vt tile through
# the DMA fabric; totals: DVE 5120 cols, PE 11264 cols (measured rates:
# DVE stt 1.27ns/col x5 + packed tt 0.6, PE 2.92ns/col).
K2_SPLITS = {(0, 3): 1024, (0, 2): 0, (1, 3): 2048, (0, 0): 0,
             (0, 1): 0, (1, 1): 1024, (1, 2): 0, (1, 0): 1024}
# vt DMA order = order tiles are needed.
K2_LOAD_ORDER = [(0, 3), (0, 2), (1, 3), (0, 0), (0, 1), (1, 1), (1, 2), (1, 0)]
# fine-grained emission schedule: per-engine instruction streams follow
# emission order, so ACT taps/scales/drains are interleaved by their
# expected ready-times (a blocked drain stalls everything behind it on the
# in-order ACT queue).  "pe" = matmuls + drain + out-DMA; "taps" = tap0 +
# tap6 prescale; "t0"/"scr" split those for (1,3) so its tap0 lands before
# the (0,2) drain blocks the queue.
K2_SEQ = [
    ("taps", (0, 3)), ("chain", (0, 3)), ("pe", (0, 3)), ("out", (0, 3)),
    ("t0", (1, 3)),
    ("pe", (0, 2)),
    ("scr", (1, 3)), ("chain", (1, 3)), ("out", (1, 3)),
    ("pe", (0, 0)),
    ("taps", (1, 1)), ("chain", (1, 1)), ("out", (1, 1)),
    ("pe", (0, 1)),
    ("pe", (1, 1)),
    ("taps", (1, 0)), ("chain", (1, 0)), ("out", (1, 0)),
    ("pe", (1, 2)),
    ("pe", (1, 0)),
]


def _build_k2(delays):
    """delays: tuple of TOPK ints (global — identical on all cores), baked
    in as static slice offsets.  Weights stay per-core inputs (wb for the
    per-partition AP scalars, dg for the PE diag stationaries) because the
    SPMD program is shared across cores while weights differ per batch.
    """
    from concourse import bacc, mybir
    from concourse.tile import TileContext

    f32 = mybir.dt.float32
    bf16 = mybir.dt.bfloat16
    Copy = mybir.ActivationFunctionType.Copy
    mult = mybir.AluOpType.mult
    add = mybir.AluOpType.add
    d = [int(x) for x in delays]
    nc = bacc.Bacc("TRN2", target_bir_lowering=False, debug=False, num_devices=NCORES)
    vt = nc.dram_tensor("vt", (BPC, C, L), bf16, kind="ExternalInput")
    # w broadcast to 128 partitions: [128, BPC*TOPK]
    wb = nc.dram_tensor("wb", (128, BPC * TOPK), f32, kind="ExternalInput")
    # [128,128] identity; the diag(w) PE stationaries are built on-device
    # by the (otherwise idle at startup) DVE — 64x less DMA than shipping
    # the diag blocks from the host
    ident = nc.dram_tensor("ident", (128, 128), bf16, kind="ExternalInput")
    ot = nc.dram_tensor("ot", (BPC, C, L), bf16, kind="ExternalOutput")

    with TileContext(nc) as tc:
        with (
            tc.tile_pool(name="consts", bufs=1) as cpool,
            tc.tile_pool(name="v", bufs=6) as vpool,
            tc.tile_pool(name="acc", bufs=2) as apool,
            tc.tile_pool(name="scr", bufs=3) as spool,
            tc.tile_pool(name="ops", bufs=3) as opool,
            tc.tile_pool(name="ps", bufs=2, space="PSUM") as pspool,
        ):
            # consts (tiny) on the SP ring right AFTER the first vt tile —
            # they gate the DVE dg-build which gates the PE's first
            # LDWEIGHTS, but vt[0] gates both engines' first real work.
            # (The ACT ring's first transfer starts ~1.6us later than SP's,
            # so everything stays on SP.)
            w_all = cpool.tile([128, BPC * TOPK], f32, name="w_all")
            id_t = cpool.tile([128, 128], bf16, name="id_t")

            # per-ring DMA bandwidth is only ~110GB/s (aggregate 228 needs
            # both rings) — lead consts split one per ring and the first
            # two vt tiles split across BOTH rings so the engines start
            # ~1.5-2.5us earlier; the ACT ring idles during the ramp
            # anyway (its out-DMAs come much later).  The ACT ring's first
            # transfer starts ~1.6us late, so it gets the smaller shares.
            vt_tiles = {}
            for gi, (b, cc) in enumerate(K2_LOAD_ORDER):
                rows = slice(128 * cc, 128 * (cc + 1))
                vt_t = vpool.tile([128, L], bf16, tag="vt", name="vt")
                if gi == 0:
                    nc.sync.dma_start(w_all[:], wb[:, :])
                    nc.scalar.dma_start(id_t[:], ident[:, :])
                    nc.sync.dma_start(vt_t[:, 0:1280], vt[b, rows, 0:1280])
                    nc.scalar.dma_start(vt_t[:, 1280:L], vt[b, rows, 1280:L])
                elif gi == 1:
                    nc.sync.dma_start(vt_t[:, 0:1024], vt[b, rows, 0:1024])
                    nc.scalar.dma_start(vt_t[:, 1024:L], vt[b, rows, 1024:L])
                else:
                    nc.sync.dma_start(vt_t[:], vt[b, rows, :])
                vt_tiles[(b, cc)] = vt_t

            def wap(b, k):
                return w_all[:, b * TOPK + k:b * TOPK + k + 1]

            dg_all = cpool.tile([128, BPC * TOPK * 128], bf16, name="dg_all")

            def dgap(b, k):
                o = (b * TOPK + k) * 128
                return dg_all[:, o:o + 128]

            # build the 14 diag stationaries on the DVE while it waits for
            # the first vt tile; the PE's very first LDWEIGHTS block
            # (lead tap of (0,3)'s u=2 chunk) goes first
            k0_first = next(k for k in range(TOPK)
                            if (d[k] + 1024) % L + 512 <= L)
            build = [(0, k0_first)] + [(0, k) for k in range(TOPK) if k != k0_first]
            build += [(1, k) for k in range(TOPK)]
            for (b, k) in build:
                nc.vector.tensor_scalar(
                    dgap(b, k), id_t[:], wap(b, k), None, mult)

            def pieces_of(k, l0, l1):
                s = (d[k] + l0) % L
                n1 = min(l1 - l0, L - s)
                out = [(0, s, n1)]
                if n1 < l1 - l0:
                    out.append((n1, (s + n1) % L, l1 - l0 - n1))
                return out

            accs = {}
            scrs = {}

            def emit_t0(b, cc, n):
                """tap0 for the whole DVE range on ACT: acc = w0 * vt_shift."""
                acc = apool.tile([128, n], bf16, tag=f"acc{n}", name=f"acc{n}")
                accs[(b, cc)] = acc
                for (po, ps, pn) in pieces_of(0, 0, n):
                    nc.scalar.activation(
                        acc[:, po:po + pn], vt_tiles[(b, cc)][:, ps:ps + pn],
                        Copy, scale=wap(b, 0))

            def emit_scr(b, cc, n):
                """taps 5+6 prescaled on ACT into col-0-aligned scratches."""
                pair = []
                for k in (5, 6):
                    scr = spool.tile([128, 2048], bf16, tag=f"scr{k}",
                                     name=f"scr{k}")
                    for (po, ps, pn) in pieces_of(k, 0, n):
                        nc.scalar.activation(
                            scr[:, po:po + pn], vt_tiles[(b, cc)][:, ps:ps + pn],
                            Copy, scale=wap(b, k))
                    pair.append(scr)
                scrs[(b, cc)] = pair

            def emit_chain(b, cc, n):
                """taps 1..4 as DVE stt, then taps 5+6 folded in with two
                2x-packed bf16 tensor_tensor adds."""
                acc = accs[(b, cc)]
                vt_t = vt_tiles[(b, cc)]
                for k in range(1, 5):
                    for (po, ps, pn) in pieces_of(k, 0, n):
                        nc.vector.scalar_tensor_tensor(
                            acc[:, po:po + pn], vt_t[:, ps:ps + pn],
                            wap(b, k), acc[:, po:po + pn], mult, add)
                for scr in scrs[(b, cc)]:
                    nc.vector.tensor_tensor(
                        acc[:, 0:n], acc[:, 0:n], scr[:, 0:n], add)

            def emit_pe(b, cc, vt_t, o_sb, us, ocol0, split_drain=False):
                # The start=True matmul must be a single full-width write
                # (a wrap-split pair with start on both pieces loses the
                # first piece), so lead each chunk with a tap that does not
                # wrap there.  u-chunks pair into [128,1024] psum tiles so
                # ACT drains 1024 wide; each pair's slice of the output
                # DMAs out right after its drain so the kernel tail only
                # exposes the final 256KB, not a whole group.
                rows = slice(128 * cc, 128 * (cc + 1))
                for pi, pair in enumerate(((0, 1), (2, 3))):
                    sub = [u for u in pair if u in us]
                    if not sub:
                        continue
                    psum = pspool.tile([128, 1024], f32,
                                       tag=f"ps{2 * pi}", name=f"ps{2 * pi}")
                    for u in sub:
                        base = 512 * (u - pair[0])
                        k0 = next(k for k in range(TOPK)
                                  if (d[k] + 512 * u) % L + 512 <= L)
                        kord = [k0] + [k for k in range(TOPK) if k != k0]
                        for j, k in enumerate(kord):
                            s = (d[k] + 512 * u) % L
                            first = (j == 0)
                            last = (j == TOPK - 1)
                            if s + 512 <= L:
                                nc.tensor.matmul(
                                    psum[:, base:base + 512], dgap(b, k),
                                    vt_t[:, s:s + 512], start=first, stop=last)
                            else:
                                n1 = L - s
                                nc.tensor.matmul(
                                    psum[:, base:base + n1], dgap(b, k),
                                    vt_t[:, s:L], start=False, stop=last)
                                nc.tensor.matmul(
                                    psum[:, base + n1:base + 512], dgap(b, k),
                                    vt_t[:, 0:512 - n1], start=False, stop=last)
                    # split_drain: the final group's tail chain (last
                    # matmul -> drain -> out-DMA) shortens when drained
                    # per 512-col u-chunk instead of per 1024 pair
                    step = 512 if split_drain else 512 * len(sub)
                    pb0 = 512 * (sub[0] - pair[0])
                    for off in range(0, 512 * len(sub), step):
                        pb = pb0 + off
                        ob = 512 * sub[0] - ocol0 + off
                        nc.scalar.activation(
                            o_sb[:, ob:ob + step], psum[:, pb:pb + step], Copy)
                        nc.scalar.dma_start(
                            ot[b, rows, ocol0 + ob:ocol0 + ob + step],
                            o_sb[:, ob:ob + step])

            for (what, (b, cc)) in K2_SEQ:
                rows = slice(128 * cc, 128 * (cc + 1))
                spl = K2_SPLITS[(b, cc)]
                if what == "taps":
                    emit_t0(b, cc, spl)
                    emit_scr(b, cc, spl)
                elif what == "t0":
                    emit_t0(b, cc, spl)
                elif what == "scr":
                    emit_scr(b, cc, spl)
                elif what == "chain":
                    emit_chain(b, cc, spl)
                elif what == "out":
                    nc.sync.dma_start(ot[b, rows, 0:spl], accs[(b, cc)][:])
                elif what == "pe":
                    o_sb = opool.tile([128, L - spl], bf16, tag=f"osb{spl}",
                                      name=f"osb{spl}")
                    emit_pe(b, cc, vt_tiles[(b, cc)], o_sb,
                            tuple(range(spl // 512, 4)), spl,
                            split_drain=((b, cc) == K2_SEQ[-1][1]))
    nc.compile()
    return nc


def _get_k1():
    if "k1" not in _CACHE:
        _CACHE["k1"] = _build_k1()
    return _CACHE["k1"]


def _get_k2(delays):
    key = ("k2", delays)
    if key not in _CACHE:
        _CACHE[key] = _build_k2(delays)
    return _CACHE[key]


_DIAG_P = np.arange(128)[:, None]
_DIAG_IDX = (np.arange(128)[:, None] + np.arange(L)[None, :]) % L


def kernel(queries, keys, values, attn_mask=None, _trace=False):
    import ml_dtypes
    from concourse import bass_utils

    f8 = ml_dtypes.float8_e4m3

    k1 = _get_k1()
    q32 = np.asarray(queries, dtype=np.float32).reshape(B, L, C)
    k32 = np.asarray(keys, dtype=np.float32).reshape(B, L, C)
    q = np.ascontiguousarray(q32.transpose(0, 2, 1).astype(f8))
    kk = np.ascontiguousarray(k32.transpose(0, 2, 1).astype(f8))

    in1 = [{"qt": q[BPC * r:BPC * (r + 1)], "kt": kk[BPC * r:BPC * (r + 1)]}
           for r in range(NCORES)]
    res1 = bass_utils.run_bass_kernel_spmd(
        k1, in1, core_ids=list(range(NCORES)), trace=_trace)
    D = np.concatenate([r["D"] for r in res1.results], axis=0).astype(np.float32)

    # selection from the fp8 correlation (rank margin >> fp8 noise)
    R = D[:, _DIAG_P, _DIAG_IDX].sum(axis=1, dtype=np.float64)  # [B, L]
    didx = np.argsort(-R.mean(axis=0), kind="stable")[:TOPK]

    # exact softmax logits for the 7 selected delays (host, fp64):
    # wlog[b,j] = (1/C) sum_{t,c} q[b,(t+d_j)%L,c] * k[b,t,c]
    q64 = q32.astype(np.float64)
    k64 = k32.astype(np.float64)
    wlog = np.empty((B, TOPK), dtype=np.float64)
    for j, dj in enumerate(didx):
        wlog[:, j] = np.einsum(
            "btc,btc->b", np.roll(q64, -int(dj), axis=1), k64) / C
    wexp = np.exp(wlog - wlog.max(axis=1, keepdims=True))
    w = (wexp / wexp.sum(axis=1, keepdims=True)).astype(np.float32)  # [B, TOPK]

    delays = tuple(int(x) for x in didx)
    v = np.ascontiguousarray(
        np.asarray(values, dtype=np.float32).reshape(B, L, C).transpose(0, 2, 1).astype(ml_dtypes.bfloat16)
    )  # [B, C, L]
    # w broadcast [128, B*TOPK] per full batch, sliced per core below
    wflat = np.ascontiguousarray(
        np.broadcast_to(w.reshape(1, B * TOPK), (128, B * TOPK)))
    ident = np.ascontiguousarray(np.eye(128, dtype=ml_dtypes.bfloat16))

    k2 = _get_k2(delays)
    in2 = []
    for r in range(NCORES):
        bsel = slice(BPC * r * TOPK, BPC * (r + 1) * TOPK)
        in2.append({
            "vt": v[BPC * r:BPC * (r + 1)],
            "wb": np.ascontiguousarray(wflat[:, bsel]),
            "ident": ident,
        })
    res2 = bass_utils.run_bass_kernel_spmd(
        k2, in2, core_ids=list(range(NCORES)), trace=_trace)
    ot = np.concatenate([r["ot"] for r in res2.results], axis=0)  # [B, C, L]
    out = ot.astype(np.float32).transpose(0, 2, 1).reshape(B, L, H, E)
    if _trace:
        kernel._last_trace = (res1, res2)
    return out


# revision 29
# speedup vs baseline: 1.0193x; 1.0193x over previous
"""DSAutoCorrelation Trainium2 kernel (v5).

Math (B=16, L=2048, H=8, E=64, C=H*E=512, top_k=7):
  R[b,l]    = sum_t <k[b,t,:], q[b,(t+l)%L,:]>_c      (= C * mean_value[b,l])
  topk over mean_b R -> 7 delays d_k; w[b,:] = softmax(R[b,d]/C)
  out[b,l,:] = sum_k w[b,k] * v[b,(l+d_k)%L,:]

Device split (8 cores, 2 batches each):
  K1 (static): D[b,p,u] = sum_{i<16,c} K^T[c,128i+p] * Q^T[c,(128i+u)%L]
      fp8 e4m3 matmuls in DoubleRow perf mode (2 channel-blocks packed per
      matmul, ~2 moving elems/cycle).  D is used ONLY for the top-k delay
      selection (fp8 noise is ~50x below the rank-7/8 margin for gaussian
      data); the 7 selected softmax logits are recomputed exactly on the
      host (0.01% of the FLOPs), so the weights carry no fp8 error.
      Wraparound via split matmuls.  kt issues on SP queue, qt on ACT
      queue; the first compute-critical pieces (kt[:, :128], qt[:, :512])
      are split out as small leading DMAs so the first matmul is not
      gated on full-tile transfers.
  K2 (lazy-compiled per delay set — delays are global, one SPMD program):
      out^T[c,l] = sum_k w_k v^T[c,(l+d_k)%L] in transposed layout.
      Three engines: PE does diag(w) matmuls for 11264 of 16384 columns
      (ACT drains PSUM pairs 1024 wide), DVE runs 4-tap stt chains for the
      other 5120, and ACT additionally computes tap 0 (Copy with
      per-partition scale) plus two pre-scaled taps that DVE folds in with
      2x-packed bf16 tensor_tensor adds (scratches are written
      col-0-aligned by ACT so the DVE adds always hit the 4B-aligned 2x
      fast path).  The diag(w) stationaries are built on-device by the
      startup-idle DVE from a [128,128] identity (64x less input DMA).
      All inputs ride the SP ring in need-order ((0,3) is a split group so
      the PE's first work needs only the first vt tile); DVE-group output
      DMAs also issue on SP so they never block ACT work, and K2_SEQ
      orders the in-order ACT queue by expected ready-time.
"""

import numpy as np

B, L, H, E = 16, 2048, 8, 64
C = H * E
NCORES = 8
BPC = B // NCORES
TOPK = 7  # int(math.log(2048))
NB = L // 128  # 16 row-blocks

_CACHE = {}


def _build_k1():
    from concourse import bacc, mybir
    from concourse.tile import TileContext

    f32 = mybir.dt.float32
    f16 = mybir.dt.float16
    f8 = mybir.dt.float8e4
    DR = mybir.MatmulPerfMode.DoubleRow
    nc = bacc.Bacc("TRN2", target_bir_lowering=False, debug=False, num_devices=NCORES)
    qt = nc.dram_tensor("qt", (BPC, C, L), f8, kind="ExternalInput")
    kt = nc.dram_tensor("kt", (BPC, C, L), f8, kind="ExternalInput")
    Dout = nc.dram_tensor("D", (BPC, 128, L), f16, kind="ExternalOutput")

    with TileContext(nc) as tc:
        with (
            tc.tile_pool(name="qk", bufs=2) as qkpool,
            tc.tile_pool(name="ps", bufs=2, space="PSUM") as pspool,
            tc.tile_pool(name="dsb", bufs=4) as dpool,
        ):
            for b in range(BPC):
                kts = []
                qts = []
                # one [128, 2, L] tile per channel-block pair; kt issues on
                # SP, qt on ACT.  For the very first pair the leading 128
                # (kt) / 512 (qt) columns go out as their own small DMAs so
                # the first LDWEIGHTS/matmul deps land early.
                for pr in range(2):
                    kt_t = qkpool.tile([128, 2, L], f8, tag=f"kt{pr}", name=f"kt{pr}")
                    qt_t = qkpool.tile([128, 2, L], f8, tag=f"qt{pr}", name=f"qt{pr}")
                    if b == 0 and pr == 0:
                        # the first i-iteration consumes ALL of qt pair 0
                        # (one 512-wide window per u) but only kt[:, :128],
                        # so stage pieces in consumption order: tiny kt/qt
                        # leads, then qt window-by-window ahead of kt bulk
                        # leads all on the SP ring — the ACT ring's first
                        # transfer starts ~1.6us later than SP's
                        for j in range(2):
                            rows = slice(128 * j, 128 * (j + 1))
                            nc.sync.dma_start(kt_t[:, j, 0:128], kt[b, rows, 0:128])
                            nc.sync.dma_start(qt_t[:, j, 0:512], qt[b, rows, 0:512])
                        for j in range(2):
                            rows = slice(128 * j, 128 * (j + 1))
                            nc.sync.dma_start(kt_t[:, j, 128:512], kt[b, rows, 128:512])
                            nc.scalar.dma_start(qt_t[:, j, 512:1024], qt[b, rows, 512:1024])
                        for j in range(2):
                            rows = slice(128 * j, 128 * (j + 1))
                            nc.scalar.dma_start(qt_t[:, j, 1024:1536], qt[b, rows, 1024:1536])
                            nc.scalar.dma_start(qt_t[:, j, 1536:L], qt[b, rows, 1536:L])
                            nc.sync.dma_start(kt_t[:, j, 512:L], kt[b, rows, 512:L])
                    else:
                        for j in range(2):
                            rows = slice(256 * pr + 128 * j, 256 * pr + 128 * (j + 1))
                            nc.sync.dma_start(kt_t[:, j, :], kt[b, rows, :])
                            nc.scalar.dma_start(qt_t[:, j, :], qt[b, rows, :])
                    kts.append(kt_t)
                    qts.append(qt_t)

                psums = [pspool.tile([128, 512], f32, tag=f"ps{u}", name=f"ps{u}") for u in range(4)]

                def mm(u, lhs, pr, i, first, last):
                    s = (128 * i + 512 * u) % L
                    if s + 512 <= L:
                        nc.tensor.matmul(
                            psums[u][:, 0:512], lhs, qts[pr][:, :, s:s + 512],
                            start=first, stop=last, perf_mode=DR)
                    else:
                        n1 = L - s
                        nc.tensor.matmul(
                            psums[u][:, 0:n1], lhs, qts[pr][:, :, s:L],
                            start=first, stop=last, perf_mode=DR)
                        nc.tensor.matmul(
                            psums[u][:, n1:512], lhs, qts[pr][:, :, 0:512 - n1],
                            start=first, stop=last, perf_mode=DR)

                # pair 0: u-inner; pair 1: u-outer with per-u stop so each
                # psum bank drains under the next u's matmuls
                for i in range(NB):
                    lhs = kts[0][:, :, 128 * i:128 * (i + 1)]
                    for u in range(4):
                        mm(u, lhs, 0, i, i == 0, False)
                for u in range(4):
                    for i in range(NB):
                        lhs = kts[1][:, :, 128 * i:128 * (i + 1)]
                        mm(u, lhs, 1, i, False, i == NB - 1)
                    d_sb = dpool.tile([128, 512], f16, tag="dsb", name="dsb")
                    if b == BPC - 1 and u == 3:
                        # the very last drain is tail-exposed: split it so
                        # the first half's DMA overlaps the second's CAST
                        for h in range(2):
                            hs = slice(256 * h, 256 * (h + 1))
                            nc.vector.tensor_copy(d_sb[:, hs], psums[u][:, hs])
                            nc.scalar.dma_start(
                                Dout[b, :, 512 * u + 256 * h:512 * u + 256 * (h + 1)],
                                d_sb[:, hs])
                    else:
                        nc.vector.tensor_copy(d_sb[:], psums[u][:])
                        nc.scalar.dma_start(Dout[b, :, 512 * u:512 * (u + 1)], d_sb[:])
    nc.compile()
    return nc


# k2 per-group column splits: group (b,cc) -> SPL; cols [0:SPL) go to the
# DVE/ACT chain, [SPL:L) to the PE (must be a multiple of 512).  (0,3) is
# split so the PE's very first work only needs the FIRST vt tile through
# the DMA fabric; totals: DVE 5120 cols, PE 11264 cols (measured rates:
# DVE stt 1.27ns/col x5 + packed tt 0.6, PE 2.92ns/col).
K2_SPLITS = {(0, 3): 1024, (0, 2): 0, (1, 3): 2048, (0, 0): 0,
             (0, 1): 0, (1, 1): 1024, (1, 2): 0, (1, 0): 1024}
# vt DMA order = order tiles are needed.
K2_LOAD_ORDER = [(0, 3), (0, 2), (1, 3), (0, 0), (0, 1), (1, 1), (1, 2), (1, 0)]
# fine-grained emission schedule: per-engine instruction streams follow
# emission order, so ACT taps/scales/drains are interleaved by their
# expected ready-times (a blocked drain stalls everything behind it on the
# in-order ACT queue).  "pe" = matmuls + drain + out-DMA; "taps" = tap0 +
# tap6 prescale; "t0"/"scr" split those for (1,3) so its tap0 lands before
# the (0,2) drain blocks the queue.
K2_SEQ = [
    ("taps", (0, 3)), ("chain", (0, 3)), ("pe", (0, 3)), ("out", (0, 3)),
    ("t0", (1, 3)),
    ("pe", (0, 2)),
    ("scr", (1, 3)), ("chain", (1, 3)), ("out", (1, 3)),
    ("pe", (0, 0)),
    ("taps", (1, 1)), ("chain", (1, 1)), ("out", (1, 1)),
    ("pe", (0, 1)),
    ("pe", (1, 1)),
    ("taps", (1, 0)), ("chain", (1, 0)), ("out", (1, 0)),
    ("pe", (1, 2)),
    ("pe", (1, 0)),
]


def _build_k2(delays):
    """delays: tuple of TOPK ints (global — identical on all cores), baked
    in as static slice offsets.  Weights stay per-core inputs (wb for the
    per-partition AP scalars, dg for the PE diag stationaries) because the
    SPMD program is shared across cores while weights differ per batch.
    """
    from concourse import bacc, mybir
    from concourse.tile import TileContext

    f32 = mybir.dt.float32
    bf16 = mybir.dt.bfloat16
    Copy = mybir.ActivationFunctionType.Copy
    mult = mybir.AluOpType.mult
    add = mybir.AluOpType.add
    d = [int(x) for x in delays]
    nc = bacc.Bacc("TRN2", target_bir_lowering=False, debug=False, num_devices=NCORES)
    vt = nc.dram_tensor("vt", (BPC, C, L), bf16, kind="ExternalInput")
    # w broadcast to 128 partitions: [128, BPC*TOPK]
    wb = nc.dram_tensor("wb", (128, BPC * TOPK), f32, kind="ExternalInput")
    # [128,128] identity; the diag(w) PE stationaries are built on-device
    # by the (otherwise idle at startup) DVE — 64x less DMA than shipping
    # the diag blocks from the host
    ident = nc.dram_tensor("ident", (128, 128), bf16, kind="ExternalInput")
    ot = nc.dram_tensor("ot", (BPC, C, L), bf16, kind="ExternalOutput")

    with TileContext(nc) as tc:
        with (
            tc.tile_pool(name="consts", bufs=1) as cpool,
            tc.tile_pool(name="v", bufs=6) as vpool,
            tc.tile_pool(name="acc", bufs=2) as apool,
            tc.tile_pool(name="scr", bufs=3) as spool,
            tc.tile_pool(name="ops", bufs=3) as opool,
            tc.tile_pool(name="ps", bufs=2, space="PSUM") as pspool,
        ):
            # consts (tiny) on the SP ring right AFTER the first vt tile —
            # they gate the DVE dg-build which gates the PE's first
            # LDWEIGHTS, but vt[0] gates both engines' first real work.
            # (The ACT ring's first transfer starts ~1.6us later than SP's,
            # so everything stays on SP.)
            w_all = cpool.tile([128, BPC * TOPK], f32, name="w_all")
            id_t = cpool.tile([128, 128], bf16, name="id_t")

            # per-ring DMA bandwidth is only ~110GB/s (aggregate 228 needs
            # both rings) — lead consts split one per ring and the first
            # two vt tiles split across BOTH rings so the engines start
            # ~1.5-2.5us earlier; the ACT ring idles during the ramp
            # anyway (its out-DMAs come much later).  The ACT ring's first
            # transfer starts ~1.6us late, so it gets the smaller shares.
            vt_tiles = {}
            for gi, (b, cc) in enumerate(K2_LOAD_ORDER):
                rows = slice(128 * cc, 128 * (cc + 1))
                vt_t = vpool.tile([128, L], bf16, tag="vt", name="vt")
                if gi == 0:
                    nc.sync.dma_start(w_all[:], wb[:, :])
                    nc.scalar.dma_start(id_t[:], ident[:, :])
                    nc.sync.dma_start(vt_t[:, 0:1280], vt[b, rows, 0:1280])
                    nc.scalar.dma_start(vt_t[:, 1280:L], vt[b, rows, 1280:L])
                elif gi == 1:
                    nc.sync.dma_start(vt_t[:, 0:1024], vt[b, rows, 0:1024])
                    nc.scalar.dma_start(vt_t[:, 1024:L], vt[b, rows, 1024:L])
                else:
                    nc.sync.dma_start(vt_t[:], vt[b, rows, :])
                vt_tiles[(b, cc)] = vt_t

            def wap(b, k):
                return w_all[:, b * TOPK + k:b * TOPK + k + 1]

            dg_all = cpool.tile([128, BPC * TOPK * 128], bf16, name="dg_all")

            def dgap(b, k):
                o = (b * TOPK + k) * 128
                return dg_all[:, o:o + 128]

            # build the 14 diag stationaries on the DVE while it waits for
            # the first vt tile; the PE's very first LDWEIGHTS block
            # (lead tap of (0,3)'s u=2 chunk) goes first
            k0_first = next(k for k in range(TOPK)
                            if (d[k] + 1024) % L + 512 <= L)
            build = [(0, k0_first)] + [(0, k) for k in range(TOPK) if k != k0_first]
            build += [(1, k) for k in range(TOPK)]
            for (b, k) in build:
                nc.vector.tensor_scalar(
                    dgap(b, k), id_t[:], wap(b, k), None, mult)

            def pieces_of(k, l0, l1):
                s = (d[k] + l0) % L
                n1 = min(l1 - l0, L - s)
                out = [(0, s, n1)]
                if n1 < l1 - l0:
                    out.append((n1, (s + n1) % L, l1 - l0 - n1))
                return out

            accs = {}
            scrs = {}

            def emit_t0(b, cc, n):
                """tap0 for the whole DVE range on ACT: acc = w0 * vt_shift."""
                acc = apool.tile([128, n], bf16, tag=f"acc{n}", name=f"acc{n}")
                accs[(b, cc)] = acc
                for (po, ps, pn) in pieces_of(0, 0, n):
                    nc.scalar.activation(
                        acc[:, po:po + pn], vt_tiles[(b, cc)][:, ps:ps + pn],
                        Copy, scale=wap(b, 0))

            def emit_scr(b, cc, n):
                """taps 5+6 prescaled on ACT into col-0-aligned scratches."""
                pair = []
                for k in (5, 6):
                    scr = spool.tile([128, 2048], bf16, tag=f"scr{k}",
                                     name=f"scr{k}")
                    for (po, ps, pn) in pieces_of(k, 0, n):
                        nc.scalar.activation(
                            scr[:, po:po + pn], vt_tiles[(b, cc)][:, ps:ps + pn],
                            Copy, scale=wap(b, k))
                    pair.append(scr)
                scrs[(b, cc)] = pair

            def emit_chain(b, cc, n):
                """taps 1..4 as DVE stt, then taps 5+6 folded in with two
                2x-packed bf16 tensor_tensor adds."""
                acc = accs[(b, cc)]
                vt_t = vt_tiles[(b, cc)]
                for k in range(1, 5):
                    for (po, ps, pn) in pieces_of(k, 0, n):
                        nc.vector.scalar_tensor_tensor(
                            acc[:, po:po + pn], vt_t[:, ps:ps + pn],
                            wap(b, k), acc[:, po:po + pn], mult, add)
                for scr in scrs[(b, cc)]:
                    nc.vector.tensor_tensor(
                        acc[:, 0:n], acc[:, 0:n], scr[:, 0:n], add)

            def emit_pe(b, cc, vt_t, o_sb, us, ocol0, split_drain=False):
                # The start=True matmul must be a single full-width write
                # (a wrap-split pair with start on both pieces loses the
                # first piece), so lead each chunk with a tap that does not
                # wrap there.  u-chunks pair into [128,1024] psum tiles so
                # ACT drains 1024 wide; each pair's slice of the output
                # DMAs out right after its drain so the kernel tail only
                # exposes the final 256KB, not a whole group.
                rows = slice(128 * cc, 128 * (cc + 1))
                for pi, pair in enumerate(((0, 1), (2, 3))):
                    sub = [u for u in pair if u in us]
                    if not sub:
                        continue
                    psum = pspool.tile([128, 1024], f32,
                                       tag=f"ps{2 * pi}", name=f"ps{2 * pi}")
                    for u in sub:
                        base = 512 * (u - pair[0])
                        k0 = next(k for k in range(TOPK)
                                  if (d[k] + 512 * u) % L + 512 <= L)
                        kord = [k0] + [k for k in range(TOPK) if k != k0]
                        for j, k in enumerate(kord):
                            s = (d[k] + 512 * u) % L
                            first = (j == 0)
                            last = (j == TOPK - 1)
                            if s + 512 <= L:
                                nc.tensor.matmul(
                                    psum[:, base:base + 512], dgap(b, k),
                                    vt_t[:, s:s + 512], start=first, stop=last)
                            else:
                                n1 = L - s
                                nc.tensor.matmul(
                                    psum[:, base:base + n1], dgap(b, k),
                                    vt_t[:, s:L], start=False, stop=last)
                                nc.tensor.matmul(
                                    psum[:, base + n1:base + 512], dgap(b, k),
                                    vt_t[:, 0:512 - n1], start=False, stop=last)
                    # split_drain: the final group's tail chain (last
                    # matmul -> drain -> out-DMA) shortens when drained
                    # per 512-col u-chunk instead of per 1024 pair
                    step = 512 if split_drain else 512 * len(sub)
                    pb0 = 512 * (sub[0] - pair[0])
                    for off in range(0, 512 * len(sub), step):
                        pb = pb0 + off
                        ob = 512 * sub[0] - ocol0 + off
                        nc.scalar.activation(
                            o_sb[:, ob:ob + step], psum[:, pb:pb + step], Copy)
                        nc.scalar.dma_start(
                            ot[b, rows, ocol0 + ob:ocol0 + ob + step],
                            o_sb[:, ob:ob + step])

            for (what, (b, cc)) in K2_SEQ:
                rows = slice(128 * cc, 128 * (cc + 1))
                spl = K2_SPLITS[(b, cc)]
                if what == "taps":
                    emit_t0(b, cc, spl)
                    emit_scr(b, cc, spl)
                elif what == "t0":
                    emit_t0(b, cc, spl)
                elif what == "scr":
                    emit_scr(b, cc, spl)
                elif what == "chain":
                    emit_chain(b, cc, spl)
                elif what == "out":
                    nc.sync.dma_start(ot[b, rows, 0:spl], accs[(b, cc)][:])
                elif what == "pe":
                    o_sb = opool.tile([128, L - spl], bf16, tag=f"osb{spl}",
                                      name=f"osb{spl}")
                    emit_pe(b, cc, vt_tiles[(b, cc)], o_sb,
                            tuple(range(spl // 512, 4)), spl,
                            split_drain=((b, cc) == K2_SEQ[-1][1]))
    nc.compile()
    return nc


def _get_k1():
    if "k1" not in _CACHE:
        _CACHE["k1"] = _build_k1()
    return _CACHE["k1"]


def _get_k2(delays):
    key = ("k2", delays)
    if key not in _CACHE:
        _CACHE[key] = _build_k2(delays)
    return _CACHE[key]


_DIAG_P = np.arange(128)[:, None]
_DIAG_IDX = (np.arange(128)[:, None] + np.arange(L)[None, :]) % L


def kernel(queries, keys, values, attn_mask=None, _trace=False):
    import ml_dtypes
    from concourse import bass_utils

    f8 = ml_dtypes.float8_e4m3

    k1 = _get_k1()
    q32 = np.asarray(queries, dtype=np.float32).reshape(B, L, C)
    k32 = np.asarray(keys, dtype=np.float32).reshape(B, L, C)
    q = np.ascontiguousarray(q32.transpose(0, 2, 1).astype(f8))
    kk = np.ascontiguousarray(k32.transpose(0, 2, 1).astype(f8))

    in1 = [{"qt": q[BPC * r:BPC * (r + 1)], "kt": kk[BPC * r:BPC * (r + 1)]}
           for r in range(NCORES)]
    res1 = bass_utils.run_bass_kernel_spmd(
        k1, in1, core_ids=list(range(NCORES)), trace=_trace)
    D = np.concatenate([r["D"] for r in res1.results], axis=0).astype(np.float32)

    # selection from the fp8 correlation (rank margin >> fp8 noise)
    R = D[:, _DIAG_P, _DIAG_IDX].sum(axis=1, dtype=np.float64)  # [B, L]
    didx = np.argsort(-R.mean(axis=0), kind="stable")[:TOPK]

    # exact softmax logits for the 7 selected delays (host, fp64):
    # wlog[b,j] = (1/C) sum_{t,c} q[b,(t+d_j)%L,c] * k[b,t,c]
    q64 = q32.astype(np.float64)
    k64 = k32.astype(np.float64)
    wlog = np.empty((B, TOPK), dtype=np.float64)
    for j, dj in enumerate(didx):
        wlog[:, j] = np.einsum(
            "btc,btc->b", np.roll(q64, -int(dj), axis=1), k64) / C
    wexp = np.exp(wlog - wlog.max(axis=1, keepdims=True))
    w = (wexp / wexp.sum(axis=1, keepdims=True)).astype(np.float32)  # [B, TOPK]

    delays = tuple(int(x) for x in didx)
    v = np.ascontiguousarray(
        np.asarray(values, dtype=np.float32).reshape(B, L, C).transpose(0, 2, 1).astype(ml_dtypes.bfloat16)
    )  # [B, C, L]
    # w broadcast [128, B*TOPK] per full batch, sliced per core below
    wflat = np.ascontiguousarray(
        np.broadcast_to(w.reshape(1, B * TOPK), (128, B * TOPK)))
    ident = np.ascontiguousarray(np.eye(128, dtype=ml_dtypes.bfloat16))

    k2 = _get_k2(delays)
    in2 = []
    for r in range(NCORES):
        bsel = slice(BPC * r * TOPK, BPC * (r + 1) * TOPK)
        in2.append({
            "vt": v[BPC * r:BPC * (r + 1)],
            "wb": np.ascontiguousarray(wflat[:, bsel]),
            "ident": ident,
        })
    res2 = bass_utils.run_bass_kernel_spmd(
        k2, in2, core_ids=list(range(NCORES)), trace=_trace)
    ot = np.concatenate([r["ot"] for r in res2.results], axis=0)  # [B, C, L]
    out = ot.astype(np.float32).transpose(0, 2, 1).reshape(B, L, H, E)
    if _trace:
        kernel._last_trace = (res1, res2)
    return out


# revision 30
# speedup vs baseline: 1.0207x; 1.0014x over previous
"""DSAutoCorrelation Trainium2 kernel (v5).

Math (B=16, L=2048, H=8, E=64, C=H*E=512, top_k=7):
  R[b,l]    = sum_t <k[b,t,:], q[b,(t+l)%L,:]>_c      (= C * mean_value[b,l])
  topk over mean_b R -> 7 delays d_k; w[b,:] = softmax(R[b,d]/C)
  out[b,l,:] = sum_k w[b,k] * v[b,(l+d_k)%L,:]

Device split (8 cores, 2 batches each):
  K1 (static): D[b,p,u] = sum_{i<16,c} K^T[c,128i+p] * Q^T[c,(128i+u)%L]
      fp8 e4m3 matmuls in DoubleRow perf mode (2 channel-blocks packed per
      matmul, ~2 moving elems/cycle).  D is used ONLY for the top-k delay
      selection (fp8 noise is ~50x below the rank-7/8 margin for gaussian
      data); the 7 selected softmax logits are recomputed exactly on the
      host (0.01% of the FLOPs), so the weights carry no fp8 error.
      Wraparound via split matmuls.  kt issues on SP queue, qt on ACT
      queue; the first compute-critical pieces (kt[:, :128], qt[:, :512])
      are split out as small leading DMAs so the first matmul is not
      gated on full-tile transfers.
  K2 (lazy-compiled per delay set — delays are global, one SPMD program):
      out^T[c,l] = sum_k w_k v^T[c,(l+d_k)%L] in transposed layout.
      Three engines: PE does diag(w) matmuls for 11264 of 16384 columns
      (ACT drains PSUM pairs 1024 wide), DVE runs 4-tap stt chains for the
      other 5120, and ACT additionally computes tap 0 (Copy with
      per-partition scale) plus two pre-scaled taps that DVE folds in with
      2x-packed bf16 tensor_tensor adds (scratches are written
      col-0-aligned by ACT so the DVE adds always hit the 4B-aligned 2x
      fast path).  The diag(w) stationaries are built on-device by the
      startup-idle DVE from a [128,128] identity (64x less input DMA).
      All inputs ride the SP ring in need-order ((0,3) is a split group so
      the PE's first work needs only the first vt tile); DVE-group output
      DMAs also issue on SP so they never block ACT work, and K2_SEQ
      orders the in-order ACT queue by expected ready-time.
"""

import numpy as np

B, L, H, E = 16, 2048, 8, 64
C = H * E
NCORES = 8
BPC = B // NCORES
TOPK = 7  # int(math.log(2048))
NB = L // 128  # 16 row-blocks

_CACHE = {}


def _build_k1():
    from concourse import bacc, mybir
    from concourse.tile import TileContext

    f32 = mybir.dt.float32
    f16 = mybir.dt.float16
    f8 = mybir.dt.float8e4
    DR = mybir.MatmulPerfMode.DoubleRow
    nc = bacc.Bacc("TRN2", target_bir_lowering=False, debug=False, num_devices=NCORES)
    qt = nc.dram_tensor("qt", (BPC, C, L), f8, kind="ExternalInput")
    kt = nc.dram_tensor("kt", (BPC, C, L), f8, kind="ExternalInput")
    Dout = nc.dram_tensor("D", (BPC, 128, L), f16, kind="ExternalOutput")

    with TileContext(nc) as tc:
        with (
            tc.tile_pool(name="qk", bufs=2) as qkpool,
            tc.tile_pool(name="ps", bufs=2, space="PSUM") as pspool,
            tc.tile_pool(name="dsb", bufs=4) as dpool,
        ):
            for b in range(BPC):
                kts = []
                qts = []
                # one [128, 2, L] tile per channel-block pair; kt issues on
                # SP, qt on ACT.  For the very first pair the leading 128
                # (kt) / 512 (qt) columns go out as their own small DMAs so
                # the first LDWEIGHTS/matmul deps land early.
                for pr in range(2):
                    kt_t = qkpool.tile([128, 2, L], f8, tag=f"kt{pr}", name=f"kt{pr}")
                    qt_t = qkpool.tile([128, 2, L], f8, tag=f"qt{pr}", name=f"qt{pr}")
                    if b == 0 and pr == 0:
                        # the first i-iteration consumes ALL of qt pair 0
                        # (one 512-wide window per u) but only kt[:, :128],
                        # so stage pieces in consumption order: tiny kt/qt
                        # leads, then qt window-by-window ahead of kt bulk
                        # leads all on the SP ring — the ACT ring's first
                        # transfer starts ~1.6us later than SP's
                        for j in range(2):
                            rows = slice(128 * j, 128 * (j + 1))
                            nc.sync.dma_start(kt_t[:, j, 0:128], kt[b, rows, 0:128])
                            nc.sync.dma_start(qt_t[:, j, 0:512], qt[b, rows, 0:512])
                        for j in range(2):
                            rows = slice(128 * j, 128 * (j + 1))
                            nc.sync.dma_start(kt_t[:, j, 128:512], kt[b, rows, 128:512])
                            nc.scalar.dma_start(qt_t[:, j, 512:1024], qt[b, rows, 512:1024])
                        for j in range(2):
                            rows = slice(128 * j, 128 * (j + 1))
                            nc.scalar.dma_start(qt_t[:, j, 1024:1536], qt[b, rows, 1024:1536])
                            nc.scalar.dma_start(qt_t[:, j, 1536:L], qt[b, rows, 1536:L])
                            nc.sync.dma_start(kt_t[:, j, 512:L], kt[b, rows, 512:L])
                    else:
                        for j in range(2):
                            rows = slice(256 * pr + 128 * j, 256 * pr + 128 * (j + 1))
                            nc.sync.dma_start(kt_t[:, j, :], kt[b, rows, :])
                            nc.scalar.dma_start(qt_t[:, j, :], qt[b, rows, :])
                    kts.append(kt_t)
                    qts.append(qt_t)

                psums = [pspool.tile([128, 512], f32, tag=f"ps{u}", name=f"ps{u}") for u in range(4)]

                def mm(u, lhs, pr, i, first, last):
                    s = (128 * i + 512 * u) % L
                    if s + 512 <= L:
                        nc.tensor.matmul(
                            psums[u][:, 0:512], lhs, qts[pr][:, :, s:s + 512],
                            start=first, stop=last, perf_mode=DR)
                    else:
                        n1 = L - s
                        nc.tensor.matmul(
                            psums[u][:, 0:n1], lhs, qts[pr][:, :, s:L],
                            start=first, stop=last, perf_mode=DR)
                        nc.tensor.matmul(
                            psums[u][:, n1:512], lhs, qts[pr][:, :, 0:512 - n1],
                            start=first, stop=last, perf_mode=DR)

                # pair 0: u-inner; pair 1: u-outer with per-u stop so each
                # psum bank drains under the next u's matmuls
                for i in range(NB):
                    lhs = kts[0][:, :, 128 * i:128 * (i + 1)]
                    for u in range(4):
                        mm(u, lhs, 0, i, i == 0, False)
                for u in range(4):
                    for i in range(NB):
                        lhs = kts[1][:, :, 128 * i:128 * (i + 1)]
                        mm(u, lhs, 1, i, False, i == NB - 1)
                    d_sb = dpool.tile([128, 512], f16, tag="dsb", name="dsb")
                    if b == BPC - 1 and u == 3:
                        # the very last drain is tail-exposed: split it so
                        # the first half's DMA overlaps the second's CAST
                        for h in range(2):
                            hs = slice(256 * h, 256 * (h + 1))
                            nc.vector.tensor_copy(d_sb[:, hs], psums[u][:, hs])
                            nc.scalar.dma_start(
                                Dout[b, :, 512 * u + 256 * h:512 * u + 256 * (h + 1)],
                                d_sb[:, hs])
                    else:
                        nc.vector.tensor_copy(d_sb[:], psums[u][:])
                        nc.scalar.dma_start(Dout[b, :, 512 * u:512 * (u + 1)], d_sb[:])
    nc.compile()
    return nc


# k2 per-group column splits: group (b,cc) -> SPL; cols [0:SPL) go to the
# DVE/ACT chain, [SPL:L) to the PE (must be a multiple of 512).  (0,3) is
# split so the PE's very first work only needs the FIRST vt tile through
# the DMA fabric; totals: DVE 5120 cols, PE 11264 cols (measured rates:
# DVE stt 1.27ns/col x5 + packed tt 0.6, PE 2.92ns/col).
K2_SPLITS = {(0, 3): 1024, (0, 2): 0, (1, 3): 2048, (0, 0): 0,
             (0, 1): 0, (1, 1): 1024, (1, 2): 0, (1, 0): 1024}
# vt DMA order = order tiles are needed.
K2_LOAD_ORDER = [(0, 3), (0, 2), (1, 3), (0, 0), (0, 1), (1, 1), (1, 2), (1, 0)]
# fine-grained emission schedule: per-engine instruction streams follow
# emission order, so ACT taps/scales/drains are interleaved by their
# expected ready-times (a blocked drain stalls everything behind it on the
# in-order ACT queue).  "pe" = matmuls + drain + out-DMA; "taps" = tap0 +
# tap6 prescale; "t0"/"scr" split those for (1,3) so its tap0 lands before
# the (0,2) drain blocks the queue.
K2_SEQ = [
    ("taps", (0, 3)), ("chain", (0, 3)), ("pe", (0, 3)), ("out", (0, 3)),
    ("t0", (1, 3)),
    ("pe", (0, 2)),
    ("scr", (1, 3)), ("chain", (1, 3)), ("out", (1, 3)),
    ("pe", (0, 0)),
    ("taps", (1, 1)), ("chain", (1, 1)), ("out", (1, 1)),
    ("pe", (0, 1)),
    ("pe", (1, 1)),
    ("taps", (1, 0)), ("chain", (1, 0)), ("out", (1, 0)),
    ("pe", (1, 2)),
    ("pe", (1, 0)),
]


def _build_k2(delays):
    """delays: tuple of TOPK ints (global — identical on all cores), baked
    in as static slice offsets.  Weights stay per-core inputs (wb for the
    per-partition AP scalars, dg for the PE diag stationaries) because the
    SPMD program is shared across cores while weights differ per batch.
    """
    from concourse import bacc, mybir
    from concourse.tile import TileContext

    f32 = mybir.dt.float32
    bf16 = mybir.dt.bfloat16
    Copy = mybir.ActivationFunctionType.Copy
    mult = mybir.AluOpType.mult
    add = mybir.AluOpType.add
    d = [int(x) for x in delays]
    nc = bacc.Bacc("TRN2", target_bir_lowering=False, debug=False, num_devices=NCORES)
    vt = nc.dram_tensor("vt", (BPC, C, L), bf16, kind="ExternalInput")
    # w broadcast to 128 partitions: [128, BPC*TOPK]
    wb = nc.dram_tensor("wb", (128, BPC * TOPK), f32, kind="ExternalInput")
    # [128,128] identity; the diag(w) PE stationaries are built on-device
    # by the (otherwise idle at startup) DVE — 64x less DMA than shipping
    # the diag blocks from the host
    ident = nc.dram_tensor("ident", (128, 128), bf16, kind="ExternalInput")
    ot = nc.dram_tensor("ot", (BPC, C, L), bf16, kind="ExternalOutput")

    with TileContext(nc) as tc:
        with (
            tc.tile_pool(name="consts", bufs=1) as cpool,
            tc.tile_pool(name="v", bufs=6) as vpool,
            tc.tile_pool(name="acc", bufs=2) as apool,
            tc.tile_pool(name="scr", bufs=3) as spool,
            tc.tile_pool(name="ops", bufs=3) as opool,
            tc.tile_pool(name="ps", bufs=2, space="PSUM") as pspool,
        ):
            # consts (tiny) on the SP ring right AFTER the first vt tile —
            # they gate the DVE dg-build which gates the PE's first
            # LDWEIGHTS, but vt[0] gates both engines' first real work.
            # (The ACT ring's first transfer starts ~1.6us later than SP's,
            # so everything stays on SP.)
            w_all = cpool.tile([128, BPC * TOPK], f32, name="w_all")
            id_t = cpool.tile([128, 128], bf16, name="id_t")

            # per-ring DMA bandwidth is only ~110GB/s (aggregate 228 needs
            # both rings) — lead consts split one per ring and the first
            # two vt tiles split across BOTH rings so the engines start
            # ~1.5-2.5us earlier; the ACT ring idles during the ramp
            # anyway (its out-DMAs come much later).  The ACT ring's first
            # transfer starts ~1.6us late, so it gets the smaller shares.
            vt_tiles = {}
            for gi, (b, cc) in enumerate(K2_LOAD_ORDER):
                rows = slice(128 * cc, 128 * (cc + 1))
                vt_t = vpool.tile([128, L], bf16, tag="vt", name="vt")
                if gi == 0:
                    nc.sync.dma_start(w_all[:], wb[:, :])
                    nc.scalar.dma_start(id_t[:], ident[:, :])
                    nc.sync.dma_start(vt_t[:, 0:1280], vt[b, rows, 0:1280])
                    nc.scalar.dma_start(vt_t[:, 1280:L], vt[b, rows, 1280:L])
                elif gi == 1:
                    nc.sync.dma_start(vt_t[:, 0:1024], vt[b, rows, 0:1024])
                    nc.scalar.dma_start(vt_t[:, 1024:L], vt[b, rows, 1024:L])
                else:
                    nc.sync.dma_start(vt_t[:], vt[b, rows, :])
                vt_tiles[(b, cc)] = vt_t

            def wap(b, k):
                return w_all[:, b * TOPK + k:b * TOPK + k + 1]

            dg_all = cpool.tile([128, BPC * TOPK * 128], bf16, name="dg_all")

            def dgap(b, k):
                o = (b * TOPK + k) * 128
                return dg_all[:, o:o + 128]

            # build the 14 diag stationaries on the DVE while it waits for
            # the first vt tile; the PE's very first LDWEIGHTS block
            # (lead tap of (0,3)'s u=2 chunk) goes first
            k0_first = next(k for k in range(TOPK)
                            if (d[k] + 1024) % L + 512 <= L)
            build = [(0, k0_first)] + [(0, k) for k in range(TOPK) if k != k0_first]
            build += [(1, k) for k in range(TOPK)]
            for (b, k) in build:
                nc.vector.tensor_scalar(
                    dgap(b, k), id_t[:], wap(b, k), None, mult)

            def pieces_of(k, l0, l1):
                s = (d[k] + l0) % L
                n1 = min(l1 - l0, L - s)
                out = [(0, s, n1)]
                if n1 < l1 - l0:
                    out.append((n1, (s + n1) % L, l1 - l0 - n1))
                return out

            accs = {}
            scrs = {}

            def emit_t0(b, cc, n):
                """tap0 for the whole DVE range on ACT: acc = w0 * vt_shift."""
                acc = apool.tile([128, n], bf16, tag=f"acc{n}", name=f"acc{n}")
                accs[(b, cc)] = acc
                for (po, ps, pn) in pieces_of(0, 0, n):
                    nc.scalar.activation(
                        acc[:, po:po + pn], vt_tiles[(b, cc)][:, ps:ps + pn],
                        Copy, scale=wap(b, 0))

            def emit_scr(b, cc, n):
                """taps 5+6 prescaled on ACT into col-0-aligned scratches."""
                pair = []
                for k in (5, 6):
                    scr = spool.tile([128, 2048], bf16, tag=f"scr{k}",
                                     name=f"scr{k}")
                    for (po, ps, pn) in pieces_of(k, 0, n):
                        nc.scalar.activation(
                            scr[:, po:po + pn], vt_tiles[(b, cc)][:, ps:ps + pn],
                            Copy, scale=wap(b, k))
                    pair.append(scr)
                scrs[(b, cc)] = pair

            def emit_chain(b, cc, n):
                """taps 1..4 as DVE stt, then taps 5+6 folded in with two
                2x-packed bf16 tensor_tensor adds."""
                acc = accs[(b, cc)]
                vt_t = vt_tiles[(b, cc)]
                for k in range(1, 5):
                    for (po, ps, pn) in pieces_of(k, 0, n):
                        nc.vector.scalar_tensor_tensor(
                            acc[:, po:po + pn], vt_t[:, ps:ps + pn],
                            wap(b, k), acc[:, po:po + pn], mult, add)
                for scr in scrs[(b, cc)]:
                    nc.vector.tensor_tensor(
                        acc[:, 0:n], acc[:, 0:n], scr[:, 0:n], add)

            def emit_pe(b, cc, vt_t, o_sb, us, ocol0, split_drain=False):
                # The start=True matmul must be a single full-width write
                # (a wrap-split pair with start on both pieces loses the
                # first piece), so lead each chunk with a tap that does not
                # wrap there.  u-chunks pair into [128,1024] psum tiles so
                # ACT drains 1024 wide; each pair's slice of the output
                # DMAs out right after its drain so the kernel tail only
                # exposes the final 256KB, not a whole group.
                rows = slice(128 * cc, 128 * (cc + 1))
                for pi, pair in enumerate(((0, 1), (2, 3))):
                    sub = [u for u in pair if u in us]
                    if not sub:
                        continue
                    psum = pspool.tile([128, 1024], f32,
                                       tag=f"ps{2 * pi}", name=f"ps{2 * pi}")
                    for u in sub:
                        base = 512 * (u - pair[0])
                        k0 = next(k for k in range(TOPK)
                                  if (d[k] + 512 * u) % L + 512 <= L)
                        kord = [k0] + [k for k in range(TOPK) if k != k0]
                        for j, k in enumerate(kord):
                            s = (d[k] + 512 * u) % L
                            first = (j == 0)
                            last = (j == TOPK - 1)
                            if s + 512 <= L:
                                nc.tensor.matmul(
                                    psum[:, base:base + 512], dgap(b, k),
                                    vt_t[:, s:s + 512], start=first, stop=last)
                            else:
                                n1 = L - s
                                nc.tensor.matmul(
                                    psum[:, base:base + n1], dgap(b, k),
                                    vt_t[:, s:L], start=False, stop=last)
                                nc.tensor.matmul(
                                    psum[:, base + n1:base + 512], dgap(b, k),
                                    vt_t[:, 0:512 - n1], start=False, stop=last)
                    # split_drain: the final group's tail chain (last
                    # matmul -> drain -> out-DMA) shortens when drained
                    # per 512-col u-chunk, with the out-DMAs issued from
                    # the idle SP sequencer so the 600ns DIRECT2D issues
                    # don't sit between the drains on the in-order ACT queue
                    step = 512 if split_drain else 512 * len(sub)
                    pb0 = 512 * (sub[0] - pair[0])
                    for off in range(0, 512 * len(sub), step):
                        pb = pb0 + off
                        ob = 512 * sub[0] - ocol0 + off
                        nc.scalar.activation(
                            o_sb[:, ob:ob + step], psum[:, pb:pb + step], Copy)
                        eng = nc.sync if split_drain else nc.scalar
                        eng.dma_start(
                            ot[b, rows, ocol0 + ob:ocol0 + ob + step],
                            o_sb[:, ob:ob + step])

            for (what, (b, cc)) in K2_SEQ:
                rows = slice(128 * cc, 128 * (cc + 1))
                spl = K2_SPLITS[(b, cc)]
                if what == "taps":
                    emit_t0(b, cc, spl)
                    emit_scr(b, cc, spl)
                elif what == "t0":
                    emit_t0(b, cc, spl)
                elif what == "scr":
                    emit_scr(b, cc, spl)
                elif what == "chain":
                    emit_chain(b, cc, spl)
                elif what == "out":
                    nc.sync.dma_start(ot[b, rows, 0:spl], accs[(b, cc)][:])
                elif what == "pe":
                    o_sb = opool.tile([128, L - spl], bf16, tag=f"osb{spl}",
                                      name=f"osb{spl}")
                    emit_pe(b, cc, vt_tiles[(b, cc)], o_sb,
                            tuple(range(spl // 512, 4)), spl,
                            split_drain=((b, cc) == K2_SEQ[-1][1]))
    nc.compile()
    return nc


def _get_k1():
    if "k1" not in _CACHE:
        _CACHE["k1"] = _build_k1()
    return _CACHE["k1"]


def _get_k2(delays):
    key = ("k2", delays)
    if key not in _CACHE:
        _CACHE[key] = _build_k2(delays)
    return _CACHE[key]


_DIAG_P = np.arange(128)[:, None]
_DIAG_IDX = (np.arange(128)[:, None] + np.arange(L)[None, :]) % L


def kernel(queries, keys, values, attn_mask=None, _trace=False):
    import ml_dtypes
    from concourse import bass_utils

    f8 = ml_dtypes.float8_e4m3

    k1 = _get_k1()
    q32 = np.asarray(queries, dtype=np.float32).reshape(B, L, C)
    k32 = np.asarray(keys, dtype=np.float32).reshape(B, L, C)
    q = np.ascontiguousarray(q32.transpose(0, 2, 1).astype(f8))
    kk = np.ascontiguousarray(k32.transpose(0, 2, 1).astype(f8))

    in1 = [{"qt": q[BPC * r:BPC * (r + 1)], "kt": kk[BPC * r:BPC * (r + 1)]}
           for r in range(NCORES)]
    res1 = bass_utils.run_bass_kernel_spmd(
        k1, in1, core_ids=list(range(NCORES)), trace=_trace)
    D = np.concatenate([r["D"] for r in res1.results], axis=0).astype(np.float32)

    # selection from the fp8 correlation (rank margin >> fp8 noise)
    R = D[:, _DIAG_P, _DIAG_IDX].sum(axis=1, dtype=np.float64)  # [B, L]
    didx = np.argsort(-R.mean(axis=0), kind="stable")[:TOPK]

    # exact softmax logits for the 7 selected delays (host, fp64):
    # wlog[b,j] = (1/C) sum_{t,c} q[b,(t+d_j)%L,c] * k[b,t,c]
    q64 = q32.astype(np.float64)
    k64 = k32.astype(np.float64)
    wlog = np.empty((B, TOPK), dtype=np.float64)
    for j, dj in enumerate(didx):
        wlog[:, j] = np.einsum(
            "btc,btc->b", np.roll(q64, -int(dj), axis=1), k64) / C
    wexp = np.exp(wlog - wlog.max(axis=1, keepdims=True))
    w = (wexp / wexp.sum(axis=1, keepdims=True)).astype(np.float32)  # [B, TOPK]

    delays = tuple(int(x) for x in didx)
    v = np.ascontiguousarray(
        np.asarray(values, dtype=np.float32).reshape(B, L, C).transpose(0, 2, 1).astype(ml_dtypes.bfloat16)
    )  # [B, C, L]
    # w broadcast [128, B*TOPK] per full batch, sliced per core below
    wflat = np.ascontiguousarray(
        np.broadcast_to(w.reshape(1, B * TOPK), (128, B * TOPK)))
    ident = np.ascontiguousarray(np.eye(128, dtype=ml_dtypes.bfloat16))

    k2 = _get_k2(delays)
    in2 = []
    for r in range(NCORES):
        bsel = slice(BPC * r * TOPK, BPC * (r + 1) * TOPK)
        in2.append({
            "vt": v[BPC * r:BPC * (r + 1)],
            "wb": np.ascontiguousarray(wflat[:, bsel]),
            "ident": ident,
        })
    res2 = bass_utils.run_bass_kernel_spmd(
        k2, in2, core_ids=list(range(NCORES)), trace=_trace)
    ot = np.concatenate([r["ot"] for r in res2.results], axis=0)  # [B, C, L]
    out = ot.astype(np.float32).transpose(0, 2, 1).reshape(B, L, H, E)
    if _trace:
        kernel._last_trace = (res1, res2)
    return out


# revision 31
# speedup vs baseline: 1.0313x; 1.0104x over previous
"""DSAutoCorrelation Trainium2 kernel (v5).

Math (B=16, L=2048, H=8, E=64, C=H*E=512, top_k=7):
  R[b,l]    = sum_t <k[b,t,:], q[b,(t+l)%L,:]>_c      (= C * mean_value[b,l])
  topk over mean_b R -> 7 delays d_k; w[b,:] = softmax(R[b,d]/C)
  out[b,l,:] = sum_k w[b,k] * v[b,(l+d_k)%L,:]

Device split (8 cores, 2 batches each):
  K1 (static): D[b,p,u] = sum_{i<16,c} K^T[c,128i+p] * Q^T[c,(128i+u)%L]
      fp8 e4m3 matmuls in DoubleRow perf mode (2 channel-blocks packed per
      matmul, ~2 moving elems/cycle).  D is used ONLY for the top-k delay
      selection (fp8 noise is ~50x below the rank-7/8 margin for gaussian
      data); the 7 selected softmax logits are recomputed exactly on the
      host (0.01% of the FLOPs), so the weights carry no fp8 error.
      Wraparound via split matmuls.  kt issues on SP queue, qt on ACT
      queue; the first compute-critical pieces (kt[:, :128], qt[:, :512])
      are split out as small leading DMAs so the first matmul is not
      gated on full-tile transfers.
  K2 (lazy-compiled per delay set — delays are global, one SPMD program):
      out^T[c,l] = sum_k w_k v^T[c,(l+d_k)%L] in transposed layout.
      Three engines: PE does diag(w) matmuls for 11264 of 16384 columns
      (ACT drains PSUM pairs 1024 wide), DVE runs 4-tap stt chains for the
      other 5120, and ACT additionally computes tap 0 (Copy with
      per-partition scale) plus two pre-scaled taps that DVE folds in with
      2x-packed bf16 tensor_tensor adds (scratches are written
      col-0-aligned by ACT so the DVE adds always hit the 4B-aligned 2x
      fast path).  The diag(w) stationaries are built on-device by the
      startup-idle DVE from a [128,128] identity (64x less input DMA).
      All inputs ride the SP ring in need-order ((0,3) is a split group so
      the PE's first work needs only the first vt tile); DVE-group output
      DMAs also issue on SP so they never block ACT work, and K2_SEQ
      orders the in-order ACT queue by expected ready-time.
"""

import numpy as np

B, L, H, E = 16, 2048, 8, 64
C = H * E
NCORES = 8
BPC = B // NCORES
TOPK = 7  # int(math.log(2048))
NB = L // 128  # 16 row-blocks

_CACHE = {}


def _build_k1():
    from concourse import bacc, mybir
    from concourse.tile import TileContext

    f32 = mybir.dt.float32
    f16 = mybir.dt.float16
    f8 = mybir.dt.float8e4
    DR = mybir.MatmulPerfMode.DoubleRow
    nc = bacc.Bacc("TRN2", target_bir_lowering=False, debug=False, num_devices=NCORES)
    qt = nc.dram_tensor("qt", (BPC, C, L), f8, kind="ExternalInput")
    kt = nc.dram_tensor("kt", (BPC, C, L), f8, kind="ExternalInput")
    Dout = nc.dram_tensor("D", (BPC, 128, L), f16, kind="ExternalOutput")

    with TileContext(nc) as tc:
        with (
            tc.tile_pool(name="qk", bufs=2) as qkpool,
            tc.tile_pool(name="ps", bufs=2, space="PSUM") as pspool,
            tc.tile_pool(name="dsb", bufs=4) as dpool,
        ):
            for b in range(BPC):
                kts = []
                qts = []
                # one [128, 2, L] tile per channel-block pair; kt issues on
                # SP, qt on ACT.  For the very first pair the leading 128
                # (kt) / 512 (qt) columns go out as their own small DMAs so
                # the first LDWEIGHTS/matmul deps land early.
                for pr in range(2):
                    kt_t = qkpool.tile([128, 2, L], f8, tag=f"kt{pr}", name=f"kt{pr}")
                    qt_t = qkpool.tile([128, 2, L], f8, tag=f"qt{pr}", name=f"qt{pr}")
                    if b == 0 and pr == 0:
                        # the first i-iteration consumes ALL of qt pair 0
                        # (one 512-wide window per u) but only kt[:, :128],
                        # so stage pieces in consumption order: tiny kt/qt
                        # leads, then qt window-by-window ahead of kt bulk
                        # leads all on the SP ring — the ACT ring's first
                        # transfer starts ~1.6us later than SP's
                        for j in range(2):
                            rows = slice(128 * j, 128 * (j + 1))
                            nc.sync.dma_start(kt_t[:, j, 0:128], kt[b, rows, 0:128])
                            nc.sync.dma_start(qt_t[:, j, 0:512], qt[b, rows, 0:512])
                        for j in range(2):
                            rows = slice(128 * j, 128 * (j + 1))
                            nc.sync.dma_start(kt_t[:, j, 128:512], kt[b, rows, 128:512])
                            nc.scalar.dma_start(qt_t[:, j, 512:1024], qt[b, rows, 512:1024])
                        for j in range(2):
                            rows = slice(128 * j, 128 * (j + 1))
                            nc.scalar.dma_start(qt_t[:, j, 1024:1536], qt[b, rows, 1024:1536])
                            nc.scalar.dma_start(qt_t[:, j, 1536:L], qt[b, rows, 1536:L])
                            nc.sync.dma_start(kt_t[:, j, 512:L], kt[b, rows, 512:L])
                    else:
                        for j in range(2):
                            rows = slice(256 * pr + 128 * j, 256 * pr + 128 * (j + 1))
                            nc.sync.dma_start(kt_t[:, j, :], kt[b, rows, :])
                            nc.scalar.dma_start(qt_t[:, j, :], qt[b, rows, :])
                    kts.append(kt_t)
                    qts.append(qt_t)

                psums = [pspool.tile([128, 512], f32, tag=f"ps{u}", name=f"ps{u}") for u in range(4)]

                def mm(u, lhs, pr, i, first, last):
                    s = (128 * i + 512 * u) % L
                    if s + 512 <= L:
                        nc.tensor.matmul(
                            psums[u][:, 0:512], lhs, qts[pr][:, :, s:s + 512],
                            start=first, stop=last, perf_mode=DR)
                    else:
                        n1 = L - s
                        nc.tensor.matmul(
                            psums[u][:, 0:n1], lhs, qts[pr][:, :, s:L],
                            start=first, stop=last, perf_mode=DR)
                        nc.tensor.matmul(
                            psums[u][:, n1:512], lhs, qts[pr][:, :, 0:512 - n1],
                            start=first, stop=last, perf_mode=DR)

                # pair 0: u-inner; pair 1: u-outer with per-u stop so each
                # psum bank drains under the next u's matmuls
                for i in range(NB):
                    lhs = kts[0][:, :, 128 * i:128 * (i + 1)]
                    for u in range(4):
                        mm(u, lhs, 0, i, i == 0, False)
                for u in range(4):
                    for i in range(NB):
                        lhs = kts[1][:, :, 128 * i:128 * (i + 1)]
                        mm(u, lhs, 1, i, False, i == NB - 1)
                    d_sb = dpool.tile([128, 512], f16, tag="dsb", name="dsb")
                    if b == BPC - 1 and u == 3:
                        # the very last drain is tail-exposed: split it so
                        # the first half's DMA overlaps the second's CAST,
                        # with the halves issued from different (idle)
                        # sequencer rings so the 600ns issues parallelize
                        for h, eng in enumerate((nc.scalar, nc.sync)):
                            hs = slice(256 * h, 256 * (h + 1))
                            nc.vector.tensor_copy(d_sb[:, hs], psums[u][:, hs])
                            eng.dma_start(
                                Dout[b, :, 512 * u + 256 * h:512 * u + 256 * (h + 1)],
                                d_sb[:, hs])
                    else:
                        nc.vector.tensor_copy(d_sb[:], psums[u][:])
                        nc.scalar.dma_start(Dout[b, :, 512 * u:512 * (u + 1)], d_sb[:])
    nc.compile()
    return nc


# k2 per-group column splits: group (b,cc) -> SPL; cols [0:SPL) go to the
# DVE/ACT chain, [SPL:L) to the PE (must be a multiple of 512).  (0,3) is
# split so the PE's very first work only needs the FIRST vt tile through
# the DMA fabric; totals: DVE 5120 cols, PE 11264 cols (measured rates:
# DVE stt 1.27ns/col x5 + packed tt 0.6, PE 2.92ns/col).
K2_SPLITS = {(0, 3): 1024, (0, 2): 0, (1, 3): 2048, (0, 0): 0,
             (0, 1): 0, (1, 1): 1024, (1, 2): 0, (1, 0): 1024}
# vt DMA order = order tiles are needed.
K2_LOAD_ORDER = [(0, 3), (0, 2), (1, 3), (0, 0), (0, 1), (1, 1), (1, 2), (1, 0)]
# fine-grained emission schedule: per-engine instruction streams follow
# emission order, so ACT taps/scales/drains are interleaved by their
# expected ready-times (a blocked drain stalls everything behind it on the
# in-order ACT queue).  "pe" = matmuls + drain + out-DMA; "taps" = tap0 +
# tap6 prescale; "t0"/"scr" split those for (1,3) so its tap0 lands before
# the (0,2) drain blocks the queue.
K2_SEQ = [
    ("taps", (0, 3)), ("chain", (0, 3)), ("pe", (0, 3)), ("out", (0, 3)),
    ("t0", (1, 3)),
    ("pe", (0, 2)),
    ("scr", (1, 3)), ("chain", (1, 3)), ("out", (1, 3)),
    ("pe", (0, 0)),
    ("taps", (1, 1)), ("chain", (1, 1)), ("out", (1, 1)),
    ("pe", (0, 1)),
    ("pe", (1, 1)),
    ("taps", (1, 0)), ("chain", (1, 0)), ("out", (1, 0)),
    ("pe", (1, 2)),
    ("pe", (1, 0)),
]


def _build_k2(delays):
    """delays: tuple of TOPK ints (global — identical on all cores), baked
    in as static slice offsets.  Weights stay per-core inputs (wb for the
    per-partition AP scalars, dg for the PE diag stationaries) because the
    SPMD program is shared across cores while weights differ per batch.
    """
    from concourse import bacc, mybir
    from concourse.tile import TileContext

    f32 = mybir.dt.float32
    bf16 = mybir.dt.bfloat16
    Copy = mybir.ActivationFunctionType.Copy
    mult = mybir.AluOpType.mult
    add = mybir.AluOpType.add
    d = [int(x) for x in delays]
    nc = bacc.Bacc("TRN2", target_bir_lowering=False, debug=False, num_devices=NCORES)
    vt = nc.dram_tensor("vt", (BPC, C, L), bf16, kind="ExternalInput")
    # w broadcast to 128 partitions: [128, BPC*TOPK]
    wb = nc.dram_tensor("wb", (128, BPC * TOPK), f32, kind="ExternalInput")
    # [128,128] identity; the diag(w) PE stationaries are built on-device
    # by the (otherwise idle at startup) DVE — 64x less DMA than shipping
    # the diag blocks from the host
    ident = nc.dram_tensor("ident", (128, 128), bf16, kind="ExternalInput")
    ot = nc.dram_tensor("ot", (BPC, C, L), bf16, kind="ExternalOutput")

    with TileContext(nc) as tc:
        with (
            tc.tile_pool(name="consts", bufs=1) as cpool,
            tc.tile_pool(name="v", bufs=6) as vpool,
            tc.tile_pool(name="acc", bufs=2) as apool,
            tc.tile_pool(name="scr", bufs=3) as spool,
            tc.tile_pool(name="ops", bufs=3) as opool,
            tc.tile_pool(name="ps", bufs=2, space="PSUM") as pspool,
        ):
            # consts (tiny) on the SP ring right AFTER the first vt tile —
            # they gate the DVE dg-build which gates the PE's first
            # LDWEIGHTS, but vt[0] gates both engines' first real work.
            # (The ACT ring's first transfer starts ~1.6us later than SP's,
            # so everything stays on SP.)
            w_all = cpool.tile([128, BPC * TOPK], f32, name="w_all")
            id_t = cpool.tile([128, 128], bf16, name="id_t")

            # per-ring DMA bandwidth is only ~110GB/s (aggregate 228 needs
            # both rings) — lead consts split one per ring and the first
            # two vt tiles split across BOTH rings so the engines start
            # ~1.5-2.5us earlier; the ACT ring idles during the ramp
            # anyway (its out-DMAs come much later).  The ACT ring's first
            # transfer starts ~1.6us late, so it gets the smaller shares.
            vt_tiles = {}
            for gi, (b, cc) in enumerate(K2_LOAD_ORDER):
                rows = slice(128 * cc, 128 * (cc + 1))
                vt_t = vpool.tile([128, L], bf16, tag="vt", name="vt")
                if gi == 0:
                    nc.sync.dma_start(w_all[:], wb[:, :])
                    nc.scalar.dma_start(id_t[:], ident[:, :])
                    nc.sync.dma_start(vt_t[:, 0:1280], vt[b, rows, 0:1280])
                    nc.scalar.dma_start(vt_t[:, 1280:L], vt[b, rows, 1280:L])
                elif gi == 1:
                    nc.sync.dma_start(vt_t[:, 0:1024], vt[b, rows, 0:1024])
                    nc.scalar.dma_start(vt_t[:, 1024:L], vt[b, rows, 1024:L])
                else:
                    nc.sync.dma_start(vt_t[:], vt[b, rows, :])
                vt_tiles[(b, cc)] = vt_t

            def wap(b, k):
                return w_all[:, b * TOPK + k:b * TOPK + k + 1]

            dg_all = cpool.tile([128, BPC * TOPK * 128], bf16, name="dg_all")

            def dgap(b, k):
                o = (b * TOPK + k) * 128
                return dg_all[:, o:o + 128]

            # build the 14 diag stationaries on the DVE while it waits for
            # the first vt tile; the PE's very first LDWEIGHTS block
            # (lead tap of (0,3)'s u=2 chunk) goes first
            k0_first = next(k for k in range(TOPK)
                            if (d[k] + 1024) % L + 512 <= L)
            build = [(0, k0_first)] + [(0, k) for k in range(TOPK) if k != k0_first]
            build += [(1, k) for k in range(TOPK)]
            for (b, k) in build:
                nc.vector.tensor_scalar(
                    dgap(b, k), id_t[:], wap(b, k), None, mult)

            def pieces_of(k, l0, l1):
                s = (d[k] + l0) % L
                n1 = min(l1 - l0, L - s)
                out = [(0, s, n1)]
                if n1 < l1 - l0:
                    out.append((n1, (s + n1) % L, l1 - l0 - n1))
                return out

            accs = {}
            scrs = {}

            def emit_t0(b, cc, n):
                """tap0 for the whole DVE range on ACT: acc = w0 * vt_shift."""
                acc = apool.tile([128, n], bf16, tag=f"acc{n}", name=f"acc{n}")
                accs[(b, cc)] = acc
                for (po, ps, pn) in pieces_of(0, 0, n):
                    nc.scalar.activation(
                        acc[:, po:po + pn], vt_tiles[(b, cc)][:, ps:ps + pn],
                        Copy, scale=wap(b, 0))

            def emit_scr(b, cc, n):
                """taps 5+6 prescaled on ACT into col-0-aligned scratches."""
                pair = []
                for k in (5, 6):
                    scr = spool.tile([128, 2048], bf16, tag=f"scr{k}",
                                     name=f"scr{k}")
                    for (po, ps, pn) in pieces_of(k, 0, n):
                        nc.scalar.activation(
                            scr[:, po:po + pn], vt_tiles[(b, cc)][:, ps:ps + pn],
                            Copy, scale=wap(b, k))
                    pair.append(scr)
                scrs[(b, cc)] = pair

            def emit_chain(b, cc, n):
                """taps 1..4 as DVE stt, then taps 5+6 folded in with two
                2x-packed bf16 tensor_tensor adds."""
                acc = accs[(b, cc)]
                vt_t = vt_tiles[(b, cc)]
                for k in range(1, 5):
                    for (po, ps, pn) in pieces_of(k, 0, n):
                        nc.vector.scalar_tensor_tensor(
                            acc[:, po:po + pn], vt_t[:, ps:ps + pn],
                            wap(b, k), acc[:, po:po + pn], mult, add)
                for scr in scrs[(b, cc)]:
                    nc.vector.tensor_tensor(
                        acc[:, 0:n], acc[:, 0:n], scr[:, 0:n], add)

            def emit_pe(b, cc, vt_t, o_sb, us, ocol0, split_drain=False):
                # The start=True matmul must be a single full-width write
                # (a wrap-split pair with start on both pieces loses the
                # first piece), so lead each chunk with a tap that does not
                # wrap there.  u-chunks pair into [128,1024] psum tiles so
                # ACT drains 1024 wide; each pair's slice of the output
                # DMAs out right after its drain so the kernel tail only
                # exposes the final 256KB, not a whole group.
                rows = slice(128 * cc, 128 * (cc + 1))
                for pi, pair in enumerate(((0, 1), (2, 3))):
                    sub = [u for u in pair if u in us]
                    if not sub:
                        continue
                    psum = pspool.tile([128, 1024], f32,
                                       tag=f"ps{2 * pi}", name=f"ps{2 * pi}")
                    for u in sub:
                        base = 512 * (u - pair[0])
                        k0 = next(k for k in range(TOPK)
                                  if (d[k] + 512 * u) % L + 512 <= L)
                        kord = [k0] + [k for k in range(TOPK) if k != k0]
                        for j, k in enumerate(kord):
                            s = (d[k] + 512 * u) % L
                            first = (j == 0)
                            last = (j == TOPK - 1)
                            if s + 512 <= L:
                                nc.tensor.matmul(
                                    psum[:, base:base + 512], dgap(b, k),
                                    vt_t[:, s:s + 512], start=first, stop=last)
                            else:
                                n1 = L - s
                                nc.tensor.matmul(
                                    psum[:, base:base + n1], dgap(b, k),
                                    vt_t[:, s:L], start=False, stop=last)
                                nc.tensor.matmul(
                                    psum[:, base + n1:base + 512], dgap(b, k),
                                    vt_t[:, 0:512 - n1], start=False, stop=last)
                    # split_drain: the final group's tail chain (last
                    # matmul -> drain -> out-DMA) shortens when drained
                    # per 512-col u-chunk, with the out-DMAs issued from
                    # the idle SP sequencer so the 600ns DIRECT2D issues
                    # don't sit between the drains on the in-order ACT queue
                    step = 512 if split_drain else 512 * len(sub)
                    pb0 = 512 * (sub[0] - pair[0])
                    for off in range(0, 512 * len(sub), step):
                        pb = pb0 + off
                        ob = 512 * sub[0] - ocol0 + off
                        nc.scalar.activation(
                            o_sb[:, ob:ob + step], psum[:, pb:pb + step], Copy)
                        eng = nc.sync if split_drain else nc.scalar
                        eng.dma_start(
                            ot[b, rows, ocol0 + ob:ocol0 + ob + step],
                            o_sb[:, ob:ob + step])

            for (what, (b, cc)) in K2_SEQ:
                rows = slice(128 * cc, 128 * (cc + 1))
                spl = K2_SPLITS[(b, cc)]
                if what == "taps":
                    emit_t0(b, cc, spl)
                    emit_scr(b, cc, spl)
                elif what == "t0":
                    emit_t0(b, cc, spl)
                elif what == "scr":
                    emit_scr(b, cc, spl)
                elif what == "chain":
                    emit_chain(b, cc, spl)
                elif what == "out":
                    nc.sync.dma_start(ot[b, rows, 0:spl], accs[(b, cc)][:])
                elif what == "pe":
                    o_sb = opool.tile([128, L - spl], bf16, tag=f"osb{spl}",
                                      name=f"osb{spl}")
                    emit_pe(b, cc, vt_tiles[(b, cc)], o_sb,
                            tuple(range(spl // 512, 4)), spl,
                            split_drain=((b, cc) == K2_SEQ[-1][1]))
    nc.compile()
    return nc


def _get_k1():
    if "k1" not in _CACHE:
        _CACHE["k1"] = _build_k1()
    return _CACHE["k1"]


def _get_k2(delays):
    key = ("k2", delays)
    if key not in _CACHE:
        _CACHE[key] = _build_k2(delays)
    return _CACHE[key]


_DIAG_P = np.arange(128)[:, None]
_DIAG_IDX = (np.arange(128)[:, None] + np.arange(L)[None, :]) % L


def kernel(queries, keys, values, attn_mask=None, _trace=False):
    import ml_dtypes
    from concourse import bass_utils

    f8 = ml_dtypes.float8_e4m3

    k1 = _get_k1()
    q32 = np.asarray(queries, dtype=np.float32).reshape(B, L, C)
    k32 = np.asarray(keys, dtype=np.float32).reshape(B, L, C)
    q = np.ascontiguousarray(q32.transpose(0, 2, 1).astype(f8))
    kk = np.ascontiguousarray(k32.transpose(0, 2, 1).astype(f8))

    in1 = [{"qt": q[BPC * r:BPC * (r + 1)], "kt": kk[BPC * r:BPC * (r + 1)]}
           for r in range(NCORES)]
    res1 = bass_utils.run_bass_kernel_spmd(
        k1, in1, core_ids=list(range(NCORES)), trace=_trace)
    D = np.concatenate([r["D"] for r in res1.results], axis=0).astype(np.float32)

    # selection from the fp8 correlation (rank margin >> fp8 noise)
    R = D[:, _DIAG_P, _DIAG_IDX].sum(axis=1, dtype=np.float64)  # [B, L]
    didx = np.argsort(-R.mean(axis=0), kind="stable")[:TOPK]

    # exact softmax logits for the 7 selected delays (host, fp64):
    # wlog[b,j] = (1/C) sum_{t,c} q[b,(t+d_j)%L,c] * k[b,t,c]
    q64 = q32.astype(np.float64)
    k64 = k32.astype(np.float64)
    wlog = np.empty((B, TOPK), dtype=np.float64)
    for j, dj in enumerate(didx):
        wlog[:, j] = np.einsum(
            "btc,btc->b", np.roll(q64, -int(dj), axis=1), k64) / C
    wexp = np.exp(wlog - wlog.max(axis=1, keepdims=True))
    w = (wexp / wexp.sum(axis=1, keepdims=True)).astype(np.float32)  # [B, TOPK]

    delays = tuple(int(x) for x in didx)
    v = np.ascontiguousarray(
        np.asarray(values, dtype=np.float32).reshape(B, L, C).transpose(0, 2, 1).astype(ml_dtypes.bfloat16)
    )  # [B, C, L]
    # w broadcast [128, B*TOPK] per full batch, sliced per core below
    wflat = np.ascontiguousarray(
        np.broadcast_to(w.reshape(1, B * TOPK), (128, B * TOPK)))
    ident = np.ascontiguousarray(np.eye(128, dtype=ml_dtypes.bfloat16))

    k2 = _get_k2(delays)
    in2 = []
    for r in range(NCORES):
        bsel = slice(BPC * r * TOPK, BPC * (r + 1) * TOPK)
        in2.append({
            "vt": v[BPC * r:BPC * (r + 1)],
            "wb": np.ascontiguousarray(wflat[:, bsel]),
            "ident": ident,
        })
    res2 = bass_utils.run_bass_kernel_spmd(
        k2, in2, core_ids=list(range(NCORES)), trace=_trace)
    ot = np.concatenate([r["ot"] for r in res2.results], axis=0)  # [B, C, L]
    out = ot.astype(np.float32).transpose(0, 2, 1).reshape(B, L, H, E)
    if _trace:
        kernel._last_trace = (res1, res2)
    return out
